# revision 1
# baseline (speedup 1.0000x reference)
"""Blockwise reconditioner (block-16 normalization) on 8 Trainium2 cores.

Math per row r, block g (block size 16):
    mean = mean(x[r, 16g:16g+16])
    var  = sum((x - mean)^2) / 15          (unbiased, ddof=1)
    out  = (x - mean) / sqrt(var + 1e-5) * scales[g] + shifts[g]

Implemented as out = x * a + b with per-block coefficients
    a = scales[g] / sqrt(var + eps)
    b = shifts[g] - mean * a
using raw = sum(x^2) - sum(x)^2/16, var = raw/15.

Sharding: data-parallel over rows; each of 8 cores handles a [512, 8192]
shard.  Per-core pipeline (Tile framework), per row-tile [128, 8192],
processed per 2048-column chunk:
  - DMA in (8KB/partition descriptors)
  - s1 = grouped reduce_sum(x) per 16-block      (DVE)
  - s2 = sum(x^2) per block on the TensorEngine: PE-transpose 128x128
    sub-blocks to PSUM, ACT squares PSUM->SBUF, masked fp32 matmuls
    (contraction over partitions = features) accumulate the 8 block-sums
    of each sub-block into a [128 blocks, 128 rows] PSUM tile, PE-flip
    back to row-major.  This keeps the second reduction off the DVE,
    which is the critical engine.
  - coefficient math on [128, 128] slices        (DVE + ACT sqrt)
  - apply out = x*a + b: two DVE passes (in-place), DMA out
Notes from HW measurement: DVE is the bottleneck (~130us busy/core);
GpSimd streaming contends with DVE for SBUF bandwidth (concurrent DVE
ops stretch to the GpSimd op duration), so GpSimd is left idle.  DMA
queues run in parallel at ~83us aggregate; ACT ~56us; PE ~85-97us.
Fixed overheads: ~7us to first op + ~13us kernel drain/barrier tail +
~11us first-chunk chain latency.  Measured HW exec ~158-161us/core
(allDVE fallback ~180us; first working version was 210us).  Structural
floor with this engine assignment is ~150us; the DMA roofline is ~91us.
"""

import sys

import numpy as np

for _p in ("/opt/trn_rl_repo",):
    if _p not in sys.path:
        sys.path.insert(0, _p)

import concourse.bacc as bacc
import concourse.bass as bass
import concourse.tile as tile
from concourse import mybir
from concourse.bass_utils import run_bass_kernel_spmd

F32 = mybir.dt.float32
ALU = mybir.AluOpType

N_CORES = 8
B_FULL = 4096          # total rows
N = 8192               # features
BLOCK = 16
NB = N // BLOCK        # 512 blocks
EPS = 1e-5
R = B_FULL // N_CORES  # 512 rows per core

CW = 2048              # column chunk width
# Of every APPLY_GP_DEN consecutive chunks, the first APPLY_GP_NUM get their
# apply (mul+add) on GpSimd; the rest on DVE.  (0, 1) = all-DVE.
APPLY_GP_NUM = 0
APPLY_GP_DEN = 1


def build_nc(rows: int = R, cols: int = N, cw: int = CW,
             apply_gp_num: int = APPLY_GP_NUM,
             apply_gp_den: int = APPLY_GP_DEN,
             pe_stats: bool = True,
             fast_recip: bool = False,
             use_divide: bool = False,
             pe_s1: bool = False,
             gp_pool_s1: bool = False) -> bass.Bass:
    nb = cols // BLOCK
    nrt = rows // 128
    ncc = cols // cw
    nbw = cw // BLOCK

    # Bacc (not raw Bass): its compile() pass splits multi-semaphore waits
    # into InstEventSemaphore chains — TRN2 allows at most 1 wait per
    # instruction and walrus codegen rejects more ("Too many sync wait").
    nc = bacc.Bacc("TRN2", target_bir_lowering=False, debug=False,
                   num_devices=N_CORES)
    x = nc.declare_dram_parameter("x", [rows, cols], F32, isOutput=False)
    scales = nc.declare_dram_parameter("scales", [nb], F32, isOutput=False)
    shifts = nc.declare_dram_parameter("shifts", [nb], F32, isOutput=False)
    if pe_stats:
        ident = nc.declare_dram_parameter("ident", [128, 128], F32, isOutput=False)
        # maskall[f, k*128 + g] = 1 iff g == 8k + f//16: matmul k of a chunk
        # accumulates sub-block k's 8 block-sums into output partitions
        # 8k..8k+8 (PE out base partition must be 0 — masks route instead).
        mask = nc.declare_dram_parameter(
            "maskall", [128, (cw // 128) * 128], F32, isOutput=False)
    out = nc.declare_dram_parameter("out", [rows, cols], F32, isOutput=True)

    with tile.TileContext(nc) as tc:
        with (
            tc.tile_pool(name="singles", bufs=1) as singles,
            tc.tile_pool(name="xp", bufs=3) as xp,
            tc.tile_pool(name="sqp", bufs=2) as sqp,
            tc.tile_pool(name="wsp", bufs=8) as wsp,
            tc.tile_pool(name="psA", bufs=2, space="PSUM") as psA,
            tc.tile_pool(name="psB", bufs=2, space="PSUM") as psB,
            tc.tile_pool(name="psF", bufs=2, space="PSUM") as psF,
            tc.tile_pool(name="stp2", bufs=2) as stp2,
        ):
            sc = singles.tile([128, nb], F32)
            sh = singles.tile([128, nb], F32)
            nc.gpsimd.dma_start(out=sc[:, :], in_=scales[:].partition_broadcast(128))
            nc.gpsimd.dma_start(out=sh[:, :], in_=shifts[:].partition_broadcast(128))
            eps_t = singles.tile([128, 1], F32)
            nc.vector.memset(eps_t[:, :], EPS)
            if pe_stats:
                ident_sb = singles.tile([128, 128], F32)
                mask_sb = singles.tile([128, (cw // 128) * 128], F32)
                nc.sync.dma_start(out=ident_sb[:, :], in_=ident[:, :])
                nc.sync.dma_start(out=mask_sb[:, :], in_=mask[:, :])
                if pe_s1:
                    # fp32r consumers require fp32r-rounded producers
                    mask_r = singles.tile([128, (cw // 128) * 128], F32)
                    nc.scalar.copy(
                        out=mask_r[:, :].bitcast(mybir.dt.float32r),
                        in_=mask_sb[:, :])

            spc = cw // 128  # 128-col sub-blocks per chunk
            for rt in range(nrt):
                r0 = rt * 128
                xt = xp.tile([128, cols], F32, tag="x")
                x3 = xt[:, :].rearrange("p (g b) -> p g b", b=BLOCK)

                for c in range(ncc):
                    sl = slice(c * cw, (c + 1) * cw)
                    gbsl = slice(c * nbw, (c + 1) * nbw)  # global block range
                    # per-chunk workspace: 9 slots of [128, nbw]
                    ws = wsp.tile([128, 9 * nbw], F32, tag="wsc")
                    nc.sync.dma_start(out=xt[:, sl], in_=x[r0 : r0 + 128, sl])
                    if gp_pool_s1:
                        # block MEAN via InstPool on the GpSimd engine:
                        # 1-input streaming op, frees the DVE of the s1
                        # reduce.  (pool has no gpsimd binding; call the
                        # vector method unbound with the gpsimd engine.)
                        in5 = x3[:, gbsl, :].unsqueeze(1).unsqueeze(1)
                        nc.gpsimd.add_instruction(mybir.InstPool(
                            name=nc.get_next_instruction_name(),
                            func=mybir.PoolFunctionType.avg,
                            ins=[nc.gpsimd.lower_ap(in5, opt=False)],
                            outs=[nc.gpsimd.lower_ap(ws[:, 0:nbw], opt=False)],
                        ))
                    elif not pe_s1:
                        nc.vector.tensor_reduce(
                            out=ws[:, 0 : nbw], in_=x3[:, gbsl, :],
                            op=ALU.add, axis=mybir.AxisListType.X,
                        )
                    s1 = ws[:, 0 * nbw : 1 * nbw]
                    s2 = ws[:, 1 * nbw : 2 * nbw]
                    mm = ws[:, 2 * nbw : 3 * nbw]
                    raw = ws[:, 3 * nbw : 4 * nbw]
                    sd = ws[:, 4 * nbw : 5 * nbw]
                    rstd = ws[:, 5 * nbw : 6 * nbw]
                    rscr = ws[:, 6 * nbw : 7 * nbw]
                    a = ws[:, 7 * nbw : 8 * nbw]
                    b = ws[:, 8 * nbw : 9 * nbw]
                    # First chunks of the first row-tile take the DVE path:
                    # the PE-stats chain (transpose->square->matmul->flip) has
                    # ~12us latency before the first coefficients exist, and
                    # the DVE is idle at kernel start anyway.
                    use_pe = pe_stats and (pe_s1 or not (rt == 0 and c < 2))
                    if not use_pe:
                        sq = sqp.tile([128, cw], F32, tag="sq")
                        nc.scalar.square(out=sq[:, :], in_=xt[:, sl])
                        sq3 = sq[:, :].rearrange("p (g b) -> p g b", b=BLOCK)
                        nc.vector.tensor_reduce(
                            out=s2, in_=sq3,
                            op=ALU.add, axis=mybir.AxisListType.X,
                        )
                    elif pe_s1:
                        # Both stats on PE: build an interleaved SBUF tile
                        # xs = [.. | xT_k | sqT_k | ..] (256 cols per sub-
                        # block k), then 16 accumulating fp32r matmuls with
                        # N=256 (1 cycle/row) produce [128 blocks, 256] =
                        # [s1 rows | s2 rows] in one PSUM tile; two flips
                        # return both stats to row-major.  fp32r's bf16-split
                        # adds ~1e-5 abs error (mask side is exact 0/1).
                        F32R = mybir.dt.float32r
                        xs = sqp.tile([128, 2 * cw], F32, tag="xs")
                        xs3 = xs[:, :].rearrange("p (k n) -> p k n", n=256)
                        for half in range(cw // 1024):
                            xT = psA.tile([128, 1024], F32, tag="xT")
                            for j in range(8):
                                col0 = c * cw + half * 1024 + j * 128
                                nc.tensor.transpose(
                                    xT[:, j * 128 : (j + 1) * 128],
                                    xt[:, col0 : col0 + 128],
                                    ident_sb[:, :],
                                )
                            hs = slice(half * 8, (half + 1) * 8)
                            nc.scalar.copy(
                                out=xs3[:, hs, 0:128].bitcast(F32R),
                                in_=xT[:, :])
                            nc.scalar.square(
                                out=xs3[:, hs, 128:256].bitcast(F32R),
                                in_=xT[:, :])
                        scps = psB.tile([128, 256], F32, tag="s2c")
                        for k in range(spc):
                            nc.tensor.matmul(
                                scps[:, :],
                                mask_r[:, k * 128 : (k + 1) * 128].bitcast(F32R),
                                xs[:, k * 256 : (k + 1) * 256].bitcast(F32R),
                                start=(k == 0), stop=(k == spc - 1),
                            )
                        st = stp2.tile([128, 256], F32, tag="st2")
                        nc.scalar.copy(out=st[:, :], in_=scps[:, :])
                        fp1 = psF.tile([128, 128], F32, tag="fp")
                        nc.tensor.transpose(
                            fp1[:, :], st[:, 0:128], ident_sb[:, :])
                        nc.scalar.copy(out=s1, in_=fp1[:, :])
                        fp2 = psF.tile([128, 128], F32, tag="fp")
                        nc.tensor.transpose(
                            fp2[:, :], st[:, 128:256], ident_sb[:, :])
                        nc.scalar.copy(out=s2, in_=fp2[:, :])
                    else:
                        # s2 on the TensorEngine: transpose x sub-blocks to
                        # PSUM, square PSUM->SBUF on ACT, block-sum via masked
                        # fp32 matmuls (contraction along partitions =
                        # features) accumulating into one [128, 128] PSUM
                        # tile, then flip [block, row] back to row-major.
                        sqT = sqp.tile([128, cw], F32, tag="sqT")
                        for half in range(cw // 1024):
                            xT = psA.tile([128, 1024], F32, tag="xT")
                            for j in range(8):
                                col0 = c * cw + half * 1024 + j * 128
                                nc.tensor.transpose(
                                    xT[:, j * 128 : (j + 1) * 128],
                                    xt[:, col0 : col0 + 128],
                                    ident_sb[:, :],
                                )
                            nc.scalar.square(
                                out=sqT[:, half * 1024 : (half + 1) * 1024],
                                in_=xT[:, :],
                            )
                        s2c = psB.tile([128, 128], F32, tag="s2c")
                        for k in range(spc):
                            nc.tensor.matmul(
                                s2c[:, :],
                                mask_sb[:, k * 128 : (k + 1) * 128],
                                sqT[:, k * 128 : (k + 1) * 128],
                                start=(k == 0), stop=(k == spc - 1),
                            )
                        st = stp2.tile([128, 128], F32, tag="st")
                        nc.scalar.copy(out=st[:, :], in_=s2c[:, :])
                        fp = psF.tile([128, 128], F32, tag="fp")
                        nc.tensor.transpose(fp[:, :], st[:, :], ident_sb[:, :])
                        nc.scalar.copy(out=s2, in_=fp[:, :])

                    # per-block a = scales/sqrt(var+eps), b = shifts - mean*a
                    # (with gp_pool_s1 the s1 slot holds the MEAN, not the sum)
                    s1_raw_k = -float(BLOCK) if gp_pool_s1 else -1.0 / BLOCK
                    s1_b_k = -1.0 if gp_pool_s1 else -1.0 / BLOCK
                    nc.scalar.square(out=mm, in_=s1)
                    nc.vector.scalar_tensor_tensor(
                        out=raw, in0=mm, scalar=s1_raw_k, in1=s2,
                        op0=ALU.mult, op1=ALU.add,
                    )
                    nc.scalar.activation(
                        out=sd, in_=raw,
                        func=mybir.ActivationFunctionType.Sqrt,
                        bias=eps_t[:, :], scale=1.0 / (BLOCK - 1),
                    )
                    if use_divide:
                        # single DVE op: a = scales / sd
                        nc.vector.tensor_tensor(
                            out=a, in0=sc[:, gbsl], in1=sd, op=ALU.divide)
                    else:
                        if fast_recip:
                            nc.vector.reciprocal_approx_fast(out=rstd, in_=sd)
                        else:
                            nc.vector.reciprocal_approx_accurate(
                                out=rstd, in_=sd, scratch=rscr)
                        nc.vector.tensor_mul(
                            out=a, in0=sc[:, gbsl], in1=rstd)
                    nc.vector.tensor_mul(out=rscr, in0=s1, in1=a)
                    nc.vector.scalar_tensor_tensor(
                        out=b, in0=rscr, scalar=s1_b_k, in1=sh[:, gbsl],
                        op0=ALU.mult, op1=ALU.add,
                    )

                    # apply out = x*a + b in place, then store
                    x3c = x3[:, gbsl, :]
                    a3 = a.unsqueeze(2).broadcast_to((128, nbw, BLOCK))
                    b3 = b.unsqueeze(2).broadcast_to((128, nbw, BLOCK))
                    nc.vector.tensor_mul(out=x3c, in0=x3c, in1=a3)
                    nc.vector.tensor_add(out=x3c, in0=x3c, in1=b3)
                    nc.sync.dma_start(out=out[r0 : r0 + 128, sl],
                                      in_=xt[:, sl])
    nc.compile()
    return nc


def aux_inputs(cw: int = CW) -> dict:
    """Constant tensors fed alongside the real inputs (PE-stats variant)."""
    spc = cw // 128
    maskall = np.zeros((128, spc * 128), np.float32)
    for k in range(spc):
        for f in range(128):
            maskall[f, k * 128 + 8 * k + f // BLOCK] = 1.0
    return {"ident": np.eye(128, dtype=np.float32), "maskall": maskall}


_NC_CACHE: dict = {}


def _get_nc() -> bass.Bass:
    if "nc" not in _NC_CACHE:
        _NC_CACHE["nc"] = build_nc()
    return _NC_CACHE["nc"]


def run_sharded(x, scales, shifts, trace: bool = False):
    """Run the SPMD kernel on 8 cores. Returns (out, BassKernelResults)."""
    x = np.ascontiguousarray(np.asarray(x, dtype=np.float32))
    scales = np.ascontiguousarray(np.asarray(scales, dtype=np.float32))
    shifts = np.ascontiguousarray(np.asarray(shifts, dtype=np.float32))
    assert x.shape == (B_FULL, N), x.shape
    nc = _get_nc()
    in_maps = [
        {"x": x[i * R : (i + 1) * R], "scales": scales, "shifts": shifts,
         **aux_inputs()}
        for i in range(N_CORES)
    ]
    res = run_bass_kernel_spmd(nc, in_maps, core_ids=list(range(N_CORES)), trace=trace)
    outs = [np.asarray(m["out"]) for m in res.results]
    return np.concatenate(outs, axis=0), res


def kernel(x, scales, shifts):
    out, _ = run_sharded(x, scales, shifts, trace=False)
    return out



# revision 9
# speedup vs baseline: 1.1859x; 1.1859x over previous
"""Blockwise reconditioner (block-16 normalization) on 8 Trainium2 cores.

Math per row r, block g (block size 16):
    mean = mean(x[r, 16g:16g+16])
    var  = sum((x - mean)^2) / 15          (unbiased, ddof=1)
    out  = (x - mean) / sqrt(var + 1e-5) * scales[g] + shifts[g]

Implemented as out = x * a + b with per-block coefficients
    a = scales[g] / sqrt(var + eps)
    b = shifts[g] - mean * a
using raw = sum(x^2) - 16*mean^2, var = raw/15.

bf16 data path (tolerance is 2e-2; bf16 end-to-end measures ~3.4e-3):
host casts x fp32 -> bf16, device computes in bf16/fp32, host casts the
bf16 output back to fp32.  This halves HBM traffic (DMA roofline ~94us
fp32 -> ~47us bf16) and enables the DVE 2x perf mode.

Sharding: data-parallel over rows; each of 8 cores handles a [512, 8192]
shard.  Per-core pipeline (Tile framework), per row-tile [128, 8192] in
2048-column chunks:
  - DMA in (4KB/partition descriptors, bf16)
  - mean per 16-block on GpSimd (InstPool avg, bf16 in / fp32 out) --
    keeps the reduce off the DVE (TensorReduce gets no bf16 speedup)
  - sum(x^2) per block on PE: transpose 128x128 sub-blocks to PSUM
    (bf16, 1 cycle/row), ACT squares PSUM->SBUF bf16, then 16 matmuls
    with stationary=sqT_k and moving=mask_k accumulate block sums
    directly in ROW-major [row, block] PSUM (no flip-back needed).
  - coefficient math batched per row-tile on [128, 512] tiles; a and b
    are written DUPLICATED (a[2g]=a[2g+1]=a_g) in bf16 so the apply's
    broadcast AP has a packed 2-byte last dim [stride 1, count 2] ->
    TensorTensor qualifies for the DVE 2x_1p perf mode.
  - apply out = x*a + b: two DVE passes in-place at 2x, DMA out bf16.
"""

import sys

import numpy as np
import ml_dtypes

for _p in ("/opt/trn_rl_repo",):
    if _p not in sys.path:
        sys.path.insert(0, _p)

import concourse.bacc as bacc
import concourse.bass as bass
import concourse.tile as tile
from concourse import mybir
from concourse.bass_utils import run_bass_kernel_spmd

F32 = mybir.dt.float32
BF16 = mybir.dt.bfloat16
ALU = mybir.AluOpType

N_CORES = 8
B_FULL = 4096          # total rows
N = 8192               # features
BLOCK = 16
NB = N // BLOCK        # 512 blocks
EPS = 1e-5
R = B_FULL // N_CORES  # 512 rows per core

CW = 2048              # column chunk width


def build_nc(rows: int = R, cols: int = N, cw: int = CW,
             s1_mode: str = "tree") -> bass.Bass:
    nb = cols // BLOCK          # 512 blocks
    nrt = rows // 128           # 4 row tiles
    ncc = cols // cw            # 4 chunks
    nbw = cw // BLOCK           # 128 blocks per chunk
    spc = cw // 128             # 16 sub-blocks per chunk

    nc = bacc.Bacc("TRN2", target_bir_lowering=False, debug=False,
                   num_devices=N_CORES)
    x = nc.declare_dram_parameter("x", [rows, cols], BF16, isOutput=False)
    # scn: scales [nb] fp32; shd: shifts duplicated [2*nb] fp32
    scn = nc.declare_dram_parameter("scn", [nb], F32, isOutput=False)
    shd = nc.declare_dram_parameter("shd", [2 * nb], F32, isOutput=False)
    identbf = nc.declare_dram_parameter("identbf", [128, 128], BF16,
                                        isOutput=False)
    # maskall[f, k*128 + g] = 1 iff g == 8k + f//16
    mask = nc.declare_dram_parameter("maskall", [128, spc * 128], BF16,
                                     isOutput=False)
    out = nc.declare_dram_parameter("out", [rows, cols], BF16, isOutput=True)

    with tile.TileContext(nc) as tc:
        with (
            tc.tile_pool(name="singles", bufs=1) as singles,
            tc.tile_pool(name="xp", bufs=3) as xp,
            tc.tile_pool(name="sqp", bufs=2) as sqp,
            tc.tile_pool(name="trp", bufs=2) as trp,
            tc.tile_pool(name="mst", bufs=2) as mst,
            tc.tile_pool(name="cof", bufs=2) as cof,
            tc.tile_pool(name="psA", bufs=2, space="PSUM") as psA,
            tc.tile_pool(name="psB", bufs=2, space="PSUM") as psB,
        ):
            scn_sb = singles.tile([128, nb], F32)
            shd_sb = singles.tile([128, 2 * nb], F32)
            nc.gpsimd.dma_start(out=scn_sb[:, :],
                                in_=scn[:].partition_broadcast(128))
            nc.gpsimd.dma_start(out=shd_sb[:, :],
                                in_=shd[:].partition_broadcast(128))
            eps_t = singles.tile([128, 1], F32)
            nc.vector.memset(eps_t[:, :], EPS)
            ident_sb = singles.tile([128, 128], BF16)
            mask_sb = singles.tile([128, spc * 128], BF16)
            nc.sync.dma_start(out=ident_sb[:, :], in_=identbf[:, :])
            nc.sync.dma_start(out=mask_sb[:, :], in_=mask[:, :])

            for rt in range(nrt):
                r0 = rt * 128
                xt = xp.tile([128, cols], BF16, tag="x")
                x3 = xt[:, :].rearrange("p (g b) -> p g b", b=BLOCK)
                m_t = mst.tile([128, nb], F32, tag="m")      # block means
                s2_ps = psB.tile([128, nb], F32, tag="s2")   # row-major s2

                for c in range(ncc):
                    sl = slice(c * cw, (c + 1) * cw)
                    gbsl = slice(c * nbw, (c + 1) * nbw)
                    nc.sync.dma_start(out=xt[:, sl], in_=x[r0 : r0 + 128, sl])

                    if s1_mode == "tree":
                        # block SUM via a 4-pass pairwise adder tree:
                        # passes 1-3 on GpSimd (walrus rejects InstPool and
                        # STT on the Pool engine; TENSOR_TENSOR add works),
                        # tiny pass 4 on DVE.  Keeps the 2048-elem reduce
                        # off the DVE, which TensorReduce would saturate.
                        p1 = trp.tile([128, cw // 2], F32, tag="p1")
                        p2 = trp.tile([128, cw // 4], F32, tag="p2")
                        p3 = trp.tile([128, cw // 8], F32, tag="p3")
                        src = xt[:, sl].rearrange("p (q e) -> p q e", e=2)
                        nc.gpsimd.tensor_add(
                            out=p1[:, :], in0=src[:, :, 0:1],
                            in1=src[:, :, 1:2])
                        s1v = p1[:, :].rearrange("p (q e) -> p q e", e=2)
                        nc.gpsimd.tensor_add(
                            out=p2[:, :], in0=s1v[:, :, 0:1],
                            in1=s1v[:, :, 1:2])
                        s2v = p2[:, :].rearrange("p (q e) -> p q e", e=2)
                        nc.gpsimd.tensor_add(
                            out=p3[:, :], in0=s2v[:, :, 0:1],
                            in1=s2v[:, :, 1:2])
                        s3v = p3[:, :].rearrange("p (q e) -> p q e", e=2)
                        nc.vector.tensor_add(
                            out=m_t[:, gbsl], in0=s3v[:, :, 0:1],
                            in1=s3v[:, :, 1:2])
                    else:
                        nc.vector.tensor_reduce(
                            out=m_t[:, gbsl], in_=x3[:, gbsl, :],
                            op=ALU.add, axis=mybir.AxisListType.X,
                        )

                    # s2 on PE: transpose x sub-blocks to PSUM (bf16),
                    # square PSUM->SBUF bf16 on ACT, then 16 matmuls
                    # stationary=sqT_k, moving=mask_k accumulating the
                    # row-major [row, block] sums into s2_ps[:, gbsl].
                    sqT = sqp.tile([128, cw], BF16, tag="sqT")
                    for half in range(cw // 1024):
                        xT = psA.tile([128, 1024], BF16, tag="xT")
                        for j in range(8):
                            col0 = c * cw + half * 1024 + j * 128
                            nc.tensor.transpose(
                                xT[:, j * 128 : (j + 1) * 128],
                                xt[:, col0 : col0 + 128],
                                ident_sb[:, :],
                            )
                        nc.scalar.square(
                            out=sqT[:, half * 1024 : (half + 1) * 1024],
                            in_=xT[:, :],
                        )
                    for k in range(spc):
                        nc.tensor.matmul(
                            s2_ps[:, c * nbw : (c + 1) * nbw],
                            sqT[:, k * 128 : (k + 1) * 128],
                            mask_sb[:, k * 128 : (k + 1) * 128],
                            start=(k == 0), stop=(k == spc - 1),
                        )

                # ---- per-row-tile coefficient math on [128, nb] ----
                mm = cof.tile([128, nb], F32, tag="mm")
                raw = cof.tile([128, nb], F32, tag="raw")
                sd = cof.tile([128, nb], F32, tag="sd")
                rstd = cof.tile([128, nb], F32, tag="rstd")
                a_f = cof.tile([128, nb], F32, tag="af")
                t_f = cof.tile([128, nb], F32, tag="tf")
                a_dup = cof.tile([128, 2 * nb], BF16, tag="ad")
                b_dup = cof.tile([128, 2 * nb], BF16, tag="bd")

                nc.scalar.square(out=mm[:, :], in_=m_t[:, :])
                # raw = s2 - s1^2/16   (s1 in m_t; s2 read from PSUM)
                nc.vector.scalar_tensor_tensor(
                    out=raw[:, :], in0=mm[:, :], scalar=-1.0 / BLOCK,
                    in1=s2_ps[:, :], op0=ALU.mult, op1=ALU.add,
                )
                # sd = sqrt(raw/15 + eps)
                nc.scalar.activation(
                    out=sd[:, :], in_=raw[:, :],
                    func=mybir.ActivationFunctionType.Sqrt,
                    bias=eps_t[:, :], scale=1.0 / (BLOCK - 1),
                )
                # a = scales / sd ; dup to bf16 [2*nb]
                nc.vector.reciprocal_approx_fast(out=rstd[:, :], in_=sd[:, :])
                nc.vector.tensor_mul(out=a_f[:, :], in0=scn_sb[:, :],
                                     in1=rstd[:, :])
                nc.vector.tensor_copy(
                    out=a_dup[:, :].rearrange("p (g e) -> p g e", e=2),
                    in_=a_f[:, :].unsqueeze(2).broadcast_to((128, nb, 2)),
                )
                # b = shifts - (s1/16)*a ; dup to bf16 [2*nb]
                nc.vector.tensor_mul(out=t_f[:, :], in0=m_t[:, :],
                                     in1=a_f[:, :])
                nc.vector.scalar_tensor_tensor(
                    out=b_dup[:, :].rearrange("p (g e) -> p g e", e=2),
                    in0=t_f[:, :].unsqueeze(2).broadcast_to((128, nb, 2)),
                    scalar=-1.0 / BLOCK,
                    in1=shd_sb[:, :].rearrange("p (g e) -> p g e", e=2),
                    op0=ALU.mult, op1=ALU.add,
                )

                # ---- apply out = x*a + b, in place, at DVE 2x ----
                for c in range(ncc):
                    sl = slice(c * cw, (c + 1) * cw)
                    x4 = xt[:, sl].rearrange("p (g b8 e) -> p g b8 e",
                                             b8=8, e=2)
                    ad = a_dup[:, c * 2 * nbw : (c + 1) * 2 * nbw]
                    bd = b_dup[:, c * 2 * nbw : (c + 1) * 2 * nbw]
                    a4 = (ad.rearrange("p (g e) -> p g e", e=2)
                          .unsqueeze(2).broadcast_to((128, nbw, 8, 2)))
                    b4 = (bd.rearrange("p (g e) -> p g e", e=2)
                          .unsqueeze(2).broadcast_to((128, nbw, 8, 2)))
                    nc.vector.tensor_mul(out=x4, in0=x4, in1=a4)
                    nc.vector.tensor_add(out=x4, in0=x4, in1=b4)
                    nc.sync.dma_start(out=out[r0 : r0 + 128, sl],
                                      in_=xt[:, sl])
    nc.compile()
    return nc


def aux_inputs(cw: int = CW) -> dict:
    """Constant tensors fed alongside the real inputs."""
    spc = cw // 128
    maskall = np.zeros((128, spc * 128), np.float32)
    for k in range(spc):
        for f in range(128):
            maskall[f, k * 128 + 8 * k + f // BLOCK] = 1.0
    return {
        "identbf": np.eye(128, dtype=np.float32).astype(ml_dtypes.bfloat16),
        "maskall": maskall.astype(ml_dtypes.bfloat16),
    }


_NC_CACHE: dict = {}


def _get_nc() -> bass.Bass:
    if "nc" not in _NC_CACHE:
        _NC_CACHE["nc"] = build_nc()
    return _NC_CACHE["nc"]


def run_sharded(x, scales, shifts, trace: bool = False):
    """Run the SPMD kernel on 8 cores. Returns (out, BassKernelResults)."""
    x = np.ascontiguousarray(np.asarray(x, dtype=np.float32))
    scales = np.ascontiguousarray(np.asarray(scales, dtype=np.float32))
    shifts = np.ascontiguousarray(np.asarray(shifts, dtype=np.float32))
    assert x.shape == (B_FULL, N), x.shape
    xb = x.astype(ml_dtypes.bfloat16)
    shd = np.repeat(shifts, 2).astype(np.float32)
    nc = _get_nc()
    aux = aux_inputs()
    in_maps = [
        {"x": xb[i * R : (i + 1) * R], "scn": scales, "shd": shd, **aux}
        for i in range(N_CORES)
    ]
    res = run_bass_kernel_spmd(nc, in_maps, core_ids=list(range(N_CORES)),
                               trace=trace)
    outs = [np.asarray(m["out"]).astype(np.float32) for m in res.results]
    return np.concatenate(outs, axis=0), res


def kernel(x, scales, shifts):
    out, _ = run_sharded(x, scales, shifts, trace=False)
    return out


# revision 19
# speedup vs baseline: 1.2486x; 1.0529x over previous
"""Blockwise reconditioner (block-16 normalization) on 8 Trainium2 cores.

Math per row r, block g (block size 16):
    mean = mean(x[r, 16g:16g+16])
    var  = sum((x - mean)^2) / 15          (unbiased, ddof=1)
    out  = (x - mean) / sqrt(var + 1e-5) * scales[g] + shifts[g]

Implemented as out = x * a + b with per-block coefficients
    a = scales[g] / sqrt(var + eps)
    b = shifts[g] - mean * a
using raw = sum(x^2) - 16*mean^2, var = raw/15.

bf16 data path (tolerance is 2e-2; bf16 end-to-end measures ~3.4e-3):
host casts x fp32 -> bf16, device computes in bf16/fp32, host casts the
bf16 output back to fp32.  This halves HBM traffic (DMA roofline ~94us
fp32 -> ~47us bf16) and enables the DVE 2x perf mode.

Sharding: data-parallel over rows; each of 8 cores handles a [512, 8192]
shard.  Per-core pipeline (Tile framework), per row-tile [128, 8192] in
2048-column chunks:
  - DMA in (4KB/partition descriptors, bf16)
  - mean per 16-block on GpSimd (InstPool avg, bf16 in / fp32 out) --
    keeps the reduce off the DVE (TensorReduce gets no bf16 speedup)
  - sum(x^2) per block on PE: transpose 128x128 sub-blocks to PSUM
    (bf16, 1 cycle/row), ACT squares PSUM->SBUF bf16, then 16 matmuls
    with stationary=sqT_k and moving=mask_k accumulate block sums
    directly in ROW-major [row, block] PSUM (no flip-back needed).
  - coefficient math batched per row-tile on [128, 512] tiles; a and b
    are written DUPLICATED (a[2g]=a[2g+1]=a_g) in bf16 so the apply's
    broadcast AP has a packed 2-byte last dim [stride 1, count 2] ->
    TensorTensor qualifies for the DVE 2x_1p perf mode.
  - apply out = x*a + b: two DVE passes in-place at 2x, DMA out bf16.
"""

import sys

import numpy as np
import ml_dtypes

for _p in ("/opt/trn_rl_repo",):
    if _p not in sys.path:
        sys.path.insert(0, _p)

import concourse.bacc as bacc
import concourse.bass as bass
import concourse.tile as tile
from concourse import mybir
from concourse.bass_utils import run_bass_kernel_spmd

F32 = mybir.dt.float32
BF16 = mybir.dt.bfloat16
ALU = mybir.AluOpType

N_CORES = 8
B_FULL = 4096          # total rows
N = 8192               # features
BLOCK = 16
NB = N // BLOCK        # 512 blocks
EPS = 1e-5
R = B_FULL // N_CORES  # 512 rows per core

CW = 2048              # column chunk width


def build_nc(rows: int = R, cols: int = N, cw: int = CW,
             s1_mode: str = "tree") -> bass.Bass:
    nb = cols // BLOCK          # 512 blocks
    nrt = rows // 128           # 4 row tiles
    ncc = cols // cw            # 4 chunks
    nbw = cw // BLOCK           # 128 blocks per chunk
    spc = cw // 128             # 16 sub-blocks per chunk

    nc = bacc.Bacc("TRN2", target_bir_lowering=False, debug=False,
                   num_devices=N_CORES)
    x = nc.declare_dram_parameter("x", [rows, cols], BF16, isOutput=False)
    # scn: scales [nb] fp32; shd: shifts duplicated [2*nb] fp32
    scn = nc.declare_dram_parameter("scn", [nb], F32, isOutput=False)
    shd = nc.declare_dram_parameter("shd", [2 * nb], F32, isOutput=False)
    identbf = nc.declare_dram_parameter("identbf", [128, 128], BF16,
                                        isOutput=False)
    # maskall[f, k*128 + g] = 1 iff g == 8k + f//16
    mask = nc.declare_dram_parameter("maskall", [128, spc * 128], BF16,
                                     isOutput=False)
    out = nc.declare_dram_parameter("out", [rows, cols], BF16, isOutput=True)

    with tile.TileContext(nc) as tc:
        with (
            tc.tile_pool(name="singles", bufs=1) as singles,
            tc.tile_pool(name="xp", bufs=4) as xp,
            tc.tile_pool(name="sqp", bufs=2) as sqp,
            tc.tile_pool(name="trp", bufs=2) as trp,
            tc.tile_pool(name="mst", bufs=2) as mst,
            tc.tile_pool(name="cof", bufs=2) as cof,
            tc.tile_pool(name="psA", bufs=2, space="PSUM") as psA,
            tc.tile_pool(name="psB", bufs=2, space="PSUM") as psB,
        ):
            # Constants go on the scalar/vector queues so the sync queue's
            # head starts streaming x immediately.
            scn_sb = singles.tile([128, nb], F32)
            shd_sb = singles.tile([128, 2 * nb], F32)
            nc.scalar.dma_start(out=scn_sb[:, :],
                                in_=scn[:].partition_broadcast(128))
            nc.scalar.dma_start(out=shd_sb[:, :],
                                in_=shd[:].partition_broadcast(128))
            eps_t = singles.tile([128, 1], F32)
            nc.vector.memset(eps_t[:, :], EPS)
            ident_sb = singles.tile([128, 128], BF16)
            mask_sb = singles.tile([128, spc * 128], BF16)
            nc.scalar.dma_start(out=ident_sb[:, :], in_=identbf[:, :])
            nc.scalar.dma_start(out=mask_sb[:, :], in_=mask[:, :])

            for rt in range(nrt):
                r0 = rt * 128
                xt = xp.tile([128, cols], BF16, tag="x")
                x3 = xt[:, :].rearrange("p (g b) -> p g b", b=BLOCK)
                m_t = mst.tile([128, nb], F32, tag="m")      # block sums s1
                s2_ps = psB.tile([128, nb], F32, tag="s2")   # row-major s2
                trees: dict = {}                             # c -> p3 tile

                for c in range(ncc):
                    sl = slice(c * cw, (c + 1) * cw)
                    gbsl = slice(c * nbw, (c + 1) * nbw)
                    nc.sync.dma_start(out=xt[:, sl], in_=x[r0 : r0 + 128, sl])

                    if s1_mode == "tree":
                        # block SUM via a pairwise-halves adder tree with
                        # contiguous (packed) operand APs: p1/p2 on GpSimd
                        # (walrus rejects InstPool/STT on the Pool engine;
                        # TENSOR_TENSOR add works), p3 on DVE at 2x (bf16
                        # packed), p4 back on GpSimd (its final fold has
                        # stride-2 reads, which the DVE runs slowly).  p4s
                        # are emitted after the chunk loop so the GpSimd
                        # queue never stalls waiting on a DVE p3.
                        p1 = trp.tile([128, cw // 2], BF16, tag="p1")
                        p2 = trp.tile([128, cw // 4], BF16, tag="p2")
                        p3 = trp.tile([128, cw // 8], BF16, tag="p3",
                                      name=f"p3_{rt}_{c}")
                        trees[c] = p3
                        src = x3[:, gbsl, :]
                        nc.gpsimd.tensor_add(
                            out=p1[:, :], in0=src[:, :, 0:8],
                            in1=src[:, :, 8:16])
                        s1v = p1[:, :].rearrange("p (g b) -> p g b", b=8)
                        nc.gpsimd.tensor_add(
                            out=p2[:, :], in0=s1v[:, :, 0:4],
                            in1=s1v[:, :, 4:8])
                        s2v = p2[:, :].rearrange("p (g b) -> p g b", b=4)
                        nc.vector.tensor_add(
                            out=p3[:, :], in0=s2v[:, :, 0:2],
                            in1=s2v[:, :, 2:4])
                    else:
                        nc.vector.tensor_reduce(
                            out=m_t[:, gbsl], in_=x3[:, gbsl, :],
                            op=ALU.add, axis=mybir.AxisListType.X,
                        )

                    # s2 on PE: transpose x sub-blocks to PSUM (bf16),
                    # square PSUM->SBUF bf16 on ACT, then 16 matmuls
                    # stationary=sqT_k, moving=mask_k accumulating the
                    # row-major [row, block] sums into s2_ps[:, gbsl].
                    sqT = sqp.tile([128, cw], BF16, tag="sqT")
                    for half in range(cw // 1024):
                        xT = psA.tile([128, 1024], BF16, tag="xT")
                        for j in range(8):
                            col0 = c * cw + half * 1024 + j * 128
                            nc.tensor.transpose(
                                xT[:, j * 128 : (j + 1) * 128],
                                xt[:, col0 : col0 + 128],
                                ident_sb[:, :],
                            )
                        nc.scalar.square(
                            out=sqT[:, half * 1024 : (half + 1) * 1024],
                            in_=xT[:, :],
                        )
                    for k in range(spc):
                        nc.tensor.matmul(
                            s2_ps[:, c * nbw : (c + 1) * nbw],
                            sqT[:, k * 128 : (k + 1) * 128],
                            mask_sb[:, k * 128 : (k + 1) * 128],
                            start=(k == 0), stop=(k == spc - 1),
                        )

                if s1_mode == "tree":
                    # deferred tree p4: fold each chunk's [g, 2] partials
                    # into the row-tile s1 staging tile (GpSimd; stride-2
                    # reads are slow on the DVE).
                    for c in range(ncc):
                        gbsl = slice(c * nbw, (c + 1) * nbw)
                        s3v = trees[c][:, :].rearrange("p (g b) -> p g b",
                                                       b=2)
                        nc.gpsimd.tensor_add(
                            out=m_t[:, gbsl], in0=s3v[:, :, 0:1],
                            in1=s3v[:, :, 1:2])

                # ---- per-row-tile coefficient math on [128, nb] ----
                mm = cof.tile([128, nb], F32, tag="mm")
                raw = cof.tile([128, nb], F32, tag="raw")
                sd = cof.tile([128, nb], F32, tag="sd")
                rstd = cof.tile([128, nb], F32, tag="rstd")
                t_f = cof.tile([128, nb], F32, tag="tf")
                a_dup = cof.tile([128, 2 * nb], BF16, tag="ad")
                b_dup = cof.tile([128, 2 * nb], BF16, tag="bd")

                nc.scalar.square(out=mm[:, :], in_=m_t[:, :])
                # raw = s2 - s1^2/16   (s1 in m_t; s2 read from PSUM)
                nc.vector.scalar_tensor_tensor(
                    out=raw[:, :], in0=mm[:, :], scalar=-1.0 / BLOCK,
                    in1=s2_ps[:, :], op0=ALU.mult, op1=ALU.add,
                )
                # sd = sqrt(raw/15 + eps)
                nc.scalar.activation(
                    out=sd[:, :], in_=raw[:, :],
                    func=mybir.ActivationFunctionType.Sqrt,
                    bias=eps_t[:, :], scale=1.0 / (BLOCK - 1),
                )
                # a = scales / sd ; dup to bf16 [2*nb] (dup-write: the
                # input APs broadcast each block twice, out is contiguous)
                nc.vector.reciprocal_approx_fast(out=rstd[:, :], in_=sd[:, :])
                nc.vector.tensor_mul(
                    out=a_dup[:, :].rearrange("p (g e) -> p g e", e=2),
                    in0=scn_sb[:, :].unsqueeze(2).broadcast_to((128, nb, 2)),
                    in1=rstd[:, :].unsqueeze(2).broadcast_to((128, nb, 2)),
                )
                # b = shifts - (s1/16)*a ; dup to bf16 [2*nb]
                nc.vector.tensor_mul(
                    out=t_f[:, :], in0=m_t[:, :],
                    in1=a_dup[:, :].rearrange("p (g e) -> p g e", e=2)[:, :, 0:1])
                nc.vector.scalar_tensor_tensor(
                    out=b_dup[:, :].rearrange("p (g e) -> p g e", e=2),
                    in0=t_f[:, :].unsqueeze(2).broadcast_to((128, nb, 2)),
                    scalar=-1.0 / BLOCK,
                    in1=shd_sb[:, :].rearrange("p (g e) -> p g e", e=2),
                    op0=ALU.mult, op1=ALU.add,
                )

                # ---- apply out = x*a + b, in place, at DVE 2x ----
                for c in range(ncc):
                    sl = slice(c * cw, (c + 1) * cw)
                    x4 = xt[:, sl].rearrange("p (g b8 e) -> p g b8 e",
                                             b8=8, e=2)
                    ad = a_dup[:, c * 2 * nbw : (c + 1) * 2 * nbw]
                    bd = b_dup[:, c * 2 * nbw : (c + 1) * 2 * nbw]
                    a4 = (ad.rearrange("p (g e) -> p g e", e=2)
                          .unsqueeze(2).broadcast_to((128, nbw, 8, 2)))
                    b4 = (bd.rearrange("p (g e) -> p g e", e=2)
                          .unsqueeze(2).broadcast_to((128, nbw, 8, 2)))
                    nc.vector.tensor_mul(out=x4, in0=x4, in1=a4)
                    nc.vector.tensor_add(out=x4, in0=x4, in1=b4)
                    # out-DMAs ride the scalar queue: on the sync queue they
                    # would head-of-line-block the next row-tiles' input DMAs.
                    nc.scalar.dma_start(out=out[r0 : r0 + 128, sl],
                                        in_=xt[:, sl])
    nc.compile()
    return nc


def aux_inputs(cw: int = CW) -> dict:
    """Constant tensors fed alongside the real inputs."""
    spc = cw // 128
    maskall = np.zeros((128, spc * 128), np.float32)
    for k in range(spc):
        for f in range(128):
            maskall[f, k * 128 + 8 * k + f // BLOCK] = 1.0
    return {
        "identbf": np.eye(128, dtype=np.float32).astype(ml_dtypes.bfloat16),
        "maskall": maskall.astype(ml_dtypes.bfloat16),
    }


_NC_CACHE: dict = {}


def _get_nc() -> bass.Bass:
    if "nc" not in _NC_CACHE:
        _NC_CACHE["nc"] = build_nc()
    return _NC_CACHE["nc"]


def run_sharded(x, scales, shifts, trace: bool = False):
    """Run the SPMD kernel on 8 cores. Returns (out, BassKernelResults)."""
    x = np.ascontiguousarray(np.asarray(x, dtype=np.float32))
    scales = np.ascontiguousarray(np.asarray(scales, dtype=np.float32))
    shifts = np.ascontiguousarray(np.asarray(shifts, dtype=np.float32))
    assert x.shape == (B_FULL, N), x.shape
    xb = x.astype(ml_dtypes.bfloat16)
    shd = np.repeat(shifts, 2).astype(np.float32)
    nc = _get_nc()
    aux = aux_inputs()
    in_maps = [
        {"x": xb[i * R : (i + 1) * R], "scn": scales, "shd": shd, **aux}
        for i in range(N_CORES)
    ]
    res = run_bass_kernel_spmd(nc, in_maps, core_ids=list(range(N_CORES)),
                               trace=trace)
    outs = [np.asarray(m["out"]).astype(np.float32) for m in res.results]
    return np.concatenate(outs, axis=0), res


def kernel(x, scales, shifts):
    out, _ = run_sharded(x, scales, shifts, trace=False)
    return out
